# revision 39
# baseline (speedup 1.0000x reference)
"""Distributed sparse-MoE routing kernel for 8 Trainium2 NeuronCores.

Problem (hardcoded shapes): x [4, 2048, 1024] fp32, router Wg [1024, 8],
single shared expert We [1024, 1024] + be [1024], top-1 routing with
per-expert capacity 1024 (= N/E), over-capacity tokens dropped.

The reference's dispatch/combine einsums are one-hot permutations and all
E experts apply the same (We, be), so the computation collapses exactly to

    out[n] = kept_n * gate_n * (h[n] @ We + be)

where gate_n is the top-1 softmax prob and kept_n depends on the token's
global position in its expert's queue (cumulative count in token order).

Sharding: tokens split 8 ways (1024/core); Wg/We/be replicated. Each core
routes its shard locally; the only global coupling is the per-expert
token-count prefix across cores, resolved with an 8x8-value AllGather.

Schedule (per core), driven by the TRN2 cost model:
  - the AllGather has a ~15us fixed latency, so its input (per-expert
    counts of the local shard) is produced as early as possible: router
    data (ht16/htlo fp16 split, see below) is DMA'd first (with one early
    We k-pair to seed the main matmul), the rest of We behind it, and the
    collective launches ~20us in while the main matmul still runs.
  - router logits in split fp16 (the PE truncates operands to ~11
    mantissa bits; argmax flips at ~1e-5 top-2 gaps cascade through the
    capacity cutoffs): logits = h16@Wg16 + h_lo@Wg16 + 2^-12*(h16@Wg_loS)
    with h_lo = fp16(h - fp16(h)) and Wg_loS the 2^12-scaled Wg residual.
  - main [1024x1024]@[1024x1024] matmul in fp16, k-outer over passes of
    3/3/2 token tiles so each We k-chunk is consumed as it lands; PSUM
    half-tiles evict (bias folded in) as soon as their accumulation
    closes, independent of the collective.
  - the collective-input DMA is issued on the SP queue *between* the
    input loads: the DMA device drains in FIFO order, so the We chunks
    issued after it queue behind it and the AllGather launches ~21us in
    instead of after the full input drain.
  - PSUM bank discipline (real TRN2 allows one open accumulation group
    per bank): 6 banks for main-matmul half-tiles, one for router
    logits/counts (all groups sequential), one for queue positions.
  - output is stored as fp16 (host upcasts); rounding adds ~2e-4 max
    relative error against a 2e-2 budget.
"""

import numpy as np

import concourse.bass as bass
import concourse.mybir as mybir
import concourse.tile as tile
from concourse import bacc
from concourse.bass_utils import run_bass_kernel_spmd

B, S, D = 4, 2048, 1024
E = 8
N_CORES = 8
N = B * S                  # 8192 tokens total
T = N // N_CORES           # 1024 tokens per core
CAP = N // E               # capacity per expert
P = 128
NK = D // P                # 8 contraction tiles
NM = T // P                # 8 token tiles per core
HF = 512                   # main matmul free-dim half (PSUM bank)
TH = T // 2                # token half (DMA chunk)

F32 = mybir.dt.float32
F16 = mybir.dt.float16
ACT_COPY = mybir.ActivationFunctionType.Copy
ACT_EXP = mybir.ActivationFunctionType.Exp
ALU = mybir.AluOpType
AXL = mybir.AxisListType


def _ap3(view, off, dims):
    return bass.AP(view.tensor, view.offset + off, dims)


def _build_nc() -> bass.Bass:
    nc = bacc.Bacc("TRN2", target_bir_lowering=False, debug=False,
                   enable_asserts=False, num_devices=N_CORES)

    ht16_d = nc.dram_tensor("ht16", [D, T], F16, kind="ExternalInput")
    htlo_d = nc.dram_tensor("htlo", [D, T], F16, kind="ExternalInput")
    wgp16_d = nc.dram_tensor("wgp16", [D, 2 * E], F16, kind="ExternalInput")
    we16_d = nc.dram_tensor("we16", [D, D], F16, kind="ExternalInput")
    be_d = nc.dram_tensor("be", [1, D], F16, kind="ExternalInput")
    wpre_d = nc.dram_tensor("wpre", [1, N_CORES], F32, kind="ExternalInput")
    out_d = nc.dram_tensor("out", [T, D], F16, kind="ExternalOutput")

    # tri[k, m] = 1 iff k <= m (token k counts toward token m's inclusive
    # queue position); ones for cross-tile prefix counts.  One DMA.
    trione = np.concatenate(
        [np.triu(np.ones((P, P))), np.ones((P, P))], axis=1)
    trione_d = nc.inline_tensor(
        trione.astype(np.float16), name="trione_c")

    with tile.TileContext(nc) as tc:
        with (
            tc.tile_pool(name="const", bufs=1) as const,
            tc.tile_pool(name="htp", bufs=1) as htp,
            tc.tile_pool(name="wep", bufs=1) as wep,
            tc.tile_pool(name="small", bufs=1) as small,
            tc.tile_pool(name="psb", bufs=6, space="PSUM") as psb,
            tc.tile_pool(name="psr", bufs=1, space="PSUM") as psr,
            tc.tile_pool(name="outp", bufs=1) as outp,
            tc.tile_pool(name="dram", bufs=1, space="DRAM") as dram,
        ):
            ht16_sb = htp.tile([P, NK * T], F16, tag="ht16")
            htlo_sb = htp.tile([P, NK * T], F16, tag="htlo")
            we16_sb = wep.tile([P, NK * D], F16, tag="we16")
            wgp16_sb = const.tile([P, NK * 2 * E], F16, tag="wgp16")
            trione_sb = const.tile([P, 2 * P], F16, tag="trione")
            be_bc = wep.tile([P, D], F16, tag="be_bc")
            wpre_bc = const.tile([P, N_CORES], F32, tag="wpre")

            # ---------------- DMA program (SP queue, issue order) ---------

            def load_h(dst_sb, src_d, h):
                # token half h of every k-tile in one DMA (1KB rows)
                v = dst_sb[:]
                dst = _ap3(v, h * TH, [v.ap[0], [T, NK], [1, TH]])
                nc.sync.dma_start(
                    dst,
                    src_d[:, h * TH:(h + 1) * TH].rearrange(
                        "(k p) t -> p k t", p=P))

            def load_we(c0, nk):
                # k-chunk [c0, c0+nk) (2KB rows)
                v = we16_sb[:]
                dst = _ap3(v, c0 * D, [v.ap[0], [D, nk], [1, D]])
                nc.sync.dma_start(
                    dst,
                    we16_d[c0 * P:(c0 + nk) * P, :].rearrange(
                        "(k p) d -> p k d", p=P))

            load_h(ht16_sb, ht16_d, 0)
            load_h(htlo_sb, htlo_d, 0)
            nc.sync.dma_start(
                wgp16_sb[:].rearrange("p (k e) -> p k e", e=2 * E),
                wgp16_d[:, :].rearrange("(k p) e -> p k e", p=P))
            load_h(ht16_sb, ht16_d, 1)
            load_we(0, 1)
            load_we(1, 1)
            load_h(htlo_sb, htlo_d, 1)
            nc.sync.dma_start(trione_sb[:], trione_d[:, :])
            load_we(2, 1)
            load_we(3, 1)

            # ---------------- PSUM banks ---------------------------------
            # Real TRN2 PSUM allows only ONE open accumulation group per
            # bank: every accumulation below is sequential within its bank,
            # concurrent ones live in different banks (6 main + R + J).
            psr_r = psr.tile([P, HF], F32, tag="psr_r")
            plg = [psr_r[:, i * 2 * E:(i + 1) * 2 * E] for i in range(4)]
            plo = [psr_r[:, 64 + i * E:64 + (i + 1) * E] for i in range(4)]
            pcnt = psr_r[:, 96:96 + E]
            # bank J: queue-position tiles (8 sequential groups)
            psr_j = psr.tile([P, HF], F32, tag="psr_j")
            ploc = [psr_j[:, b * E:(b + 1) * E] for b in range(NM)]

            logits = small.tile([P, NM * E], F32, tag="logits")
            lgraw = [small.tile([P, 4 * 2 * E], F32, tag=f"lgraw{g}",
                                name=f"lgraw{g}") for g in range(2)]
            mask_all = small.tile([P, NM * E], F16, tag="mask")
            lmax = small.tile([P, NM], F32, tag="lmax")

            def router_hi(g):
                # 4 sequential k-inner accumulations (one open group at a
                # time in bank R), then one ACT evict to SBUF
                for i in range(4):
                    b = g * 4 + i
                    for k in range(NK):
                        nc.tensor.matmul(
                            plg[i],
                            ht16_sb[:, k * T + b * P:k * T + (b + 1) * P],
                            wgp16_sb[:, k * 2 * E:(k + 1) * 2 * E],
                            start=(k == 0), stop=(k == NK - 1),
                            skip_group_check=True)
                pv = psr_r[:]
                nc.scalar.activation(
                    lgraw[g][:], _ap3(pv, 0, [pv.ap[0], [1, 4 * 2 * E]]),
                    ACT_COPY)

            def router_lo(g):
                for i in range(4):
                    b = g * 4 + i
                    for k in range(NK):
                        nc.tensor.matmul(
                            plo[i],
                            htlo_sb[:, k * T + b * P:k * T + (b + 1) * P],
                            wgp16_sb[:, k * 2 * E:k * 2 * E + E],
                            start=(k == 0), stop=(k == NK - 1),
                            skip_group_check=True)

            def router_finish(g):
                # DVE: logits = hi@Wg16 + lo@Wg16 + 2^-12*hi@Wg_loS
                lr = lgraw[g][:]
                lg = logits[:, g * 4 * E:(g + 1) * 4 * E]
                l3w = lg.rearrange("p (b e) -> p b e", e=E)
                nc.vector.scalar_tensor_tensor(
                    l3w,
                    _ap3(lr, E, [lr.ap[0], [2 * E, 4], [1, E]]),
                    1.0 / 4096.0,
                    _ap3(lr, 0, [lr.ap[0], [2 * E, 4], [1, E]]),
                    ALU.mult, ALU.add)
                pv = psr_r[:]
                nc.vector.tensor_tensor(
                    l3w, l3w, _ap3(pv, 64, [pv.ap[0], [E, 4], [1, E]]),
                    ALU.add)
                l3 = lg.rearrange("p (b e) -> p b e", e=E)
                lm = lmax[:, g * 4:(g + 1) * 4]
                nc.vector.tensor_reduce(lm, l3, AXL.X, ALU.max)
                lmb = bass.AP(lm.tensor, lm.offset, [lm.ap[0], [1, 4], [0, E]])
                mk = mask_all[:, g * 4 * E:(g + 1) * 4 * E]
                nc.vector.tensor_tensor(
                    mk.rearrange("p (b e) -> p b e", e=E), l3, lmb,
                    ALU.is_equal)

            def counts():
                for b in range(NM):
                    nc.tensor.matmul(
                        pcnt, trione_sb[:, P:2 * P],
                        mask_all[:, b * E:(b + 1) * E],
                        start=(b == 0), stop=(b == NM - 1),
                        skip_group_check=True)

            pm = {}

            def mm_alloc(tiles):
                for t in tiles:
                    for h in range(2):
                        pm[(t, h)] = psb.tile(
                            [P, HF], F32, tag="ps", name=f"pm{t}_{h}")

            def mm_k(tiles, ks):
                for k in ks:
                    for t in tiles:
                        for h in range(2):
                            nc.tensor.matmul(
                                pm[(t, h)][:],
                                ht16_sb[:, k * T + t * P:k * T + (t + 1) * P],
                                we16_sb[:, k * D + h * HF:k * D + (h + 1) * HF],
                                start=(k == 0), stop=False,
                                skip_group_check=True)

            def mm_close(tiles, with_bias):
                # k7 closes each accumulation; optionally folds the bias in
                # via a K=1 ones (x) be matmul so evict+scale fuse later.
                for t in tiles:
                    for h in range(2):
                        nc.tensor.matmul(
                            pm[(t, h)][:],
                            ht16_sb[:, 7 * T + t * P:7 * T + (t + 1) * P],
                            we16_sb[:, 7 * D + h * HF:7 * D + (h + 1) * HF],
                            start=False, stop=not with_bias,
                            skip_group_check=True)
                        if with_bias:
                            nc.tensor.matmul(
                                pm[(t, h)][:],
                                trione_sb[0:1, P:P + P],
                                be_bc[0:1, h * HF:(h + 1) * HF],
                                start=False, stop=True,
                                skip_group_check=True)

            # --- PE issue order ---
            router_hi(0)
            router_lo(0)
            router_finish(0)
            router_hi(1)
            mm_alloc([0, 1, 2])
            mm_k([0, 1, 2], [0, 1])
            router_lo(1)
            router_finish(1)
            counts()
            # ---------------- counts AllGather ----------------------------
            # ag_in is issued on SP *between* the input loads: the DMA
            # device is a FIFO, so the loads issued after it (we k4-7,
            # wpre, be) arrive behind it and the collective input never
            # waits for the full input drain.
            cnt_sb = small.tile([1, E], F32, tag="cnt")
            nc.vector.tensor_copy(cnt_sb[:], pcnt[0:1, :])
            ag_in = dram.tile([1, E], F32)
            ag_out = dram.tile([N_CORES, E], F32, addr_space="Shared")
            nc.sync.dma_start(ag_in[:], cnt_sb[:])
            nc.gpsimd.collective_compute(
                "AllGather", ALU.bypass,
                ins=[ag_in[:].opt()],
                outs=[ag_out[:].opt()],
                replica_groups=[list(range(N_CORES))])
            load_we(4, 1)
            load_we(5, 1)
            load_we(6, 1)
            load_we(7, 1)
            wpv = wpre_d[:, :]
            nc.sync.dma_start(
                wpre_bc[:], bass.AP(wpv.tensor, wpv.offset,
                                    [[0, P], [1, N_CORES]]))
            bev = be_d[:, :]
            nc.sync.dma_start(
                be_bc[:], bass.AP(bev.tensor, bev.offset, [[0, P], [1, D]]))
            agout_bc = small.tile([P, N_CORES * E], F32, tag="agout")
            agv = ag_out[:]
            nc.sync.dma_start(
                agout_bc[:], bass.AP(agv.tensor, agv.offset,
                                     [[0, P], [1, N_CORES * E]]))
            mm_k([0, 1, 2], [2, 3, 4, 5, 6])  # we k-singles land 0.73us apart
            mm_close([0, 1, 2], with_bias=False)
            # within-shard inclusive queue positions (needed only by the
            # post-collective chain); 8 separate PSUM slots, no WAR chains
            loc_all = small.tile([P, NM * E], F32, tag="loc")
            for b in range(NM):
                nc.tensor.matmul(
                    ploc[b], trione_sb[:, 0:P], mask_all[:, b * E:(b + 1) * E],
                    start=True, stop=(b == 0), skip_group_check=True)
                for a in range(b):
                    nc.tensor.matmul(
                        ploc[b], trione_sb[:, P:2 * P],
                        mask_all[:, a * E:(a + 1) * E],
                        start=False, stop=(a == b - 1),
                        skip_group_check=True)
            nc.vector.tensor_copy(loc_all[:], psr_j[:, 0:NM * E])
            # mask-scalarized local position (pre-collective): myloc[p, b] =
            # sum_e mask * loc for the token's own expert
            mylocm = small.tile([P, NM * E], F32, tag="mylocm")
            nc.vector.tensor_tensor(
                mylocm[:], loc_all[:], mask_all[:], ALU.mult)
            myloc = small.tile([P, NM], F32, tag="myloc")
            nc.vector.tensor_reduce(
                myloc[:], mylocm[:].rearrange("p (b e) -> p b e", e=E),
                AXL.X, ALU.add)
            mm_alloc([3, 4, 5])
            mm_k([3, 4, 5], range(7))
            mm_close([3, 4, 5], with_bias=False)
            mm_alloc([6, 7])
            mm_k([6], range(7))
            mm_close([6], with_bias=True)
            mm_k([7], range(7))
            mm_close([7], with_bias=True)


            # ---------------- gate (softmax value), off critical path -----
            # a tiny early ACT op triggers the activation-table load at t~2
            actwarm = small.tile([1, 16], F32, tag="actwarm")
            nc.scalar.activation(actwarm[:], trione_sb[0:1, 0:16], ACT_COPY)
            la = logits[:]
            l3 = la.rearrange("p (b e) -> p b e", e=E)
            lm = lmax[:]
            lmax_b = bass.AP(lm.tensor, lm.offset, [lm.ap[0], [1, NM], [0, E]])
            lsub = small.tile([P, NM * E], F32, tag="lsub")
            nc.vector.tensor_tensor(
                lsub[:].rearrange("p (b e) -> p b e", e=E), l3, lmax_b,
                ALU.subtract)
            expd = small.tile([P, NM * E], F32, tag="expd")
            nc.scalar.activation(expd[:], lsub[:], ACT_EXP)
            ssum = small.tile([P, NM], F32, tag="ssum")
            nc.vector.tensor_reduce(
                ssum[:], expd[:].rearrange("p (b e) -> p b e", e=E),
                AXL.X, ALU.add)
            gate = small.tile([P, NM], F32, tag="gate")
            nc.vector.reciprocal(gate[:], ssum[:])

            # ---------------- evict / scale / store -----------------------
            ot = [outp.tile([P, D], F16, tag=f"ot{t}", name=f"ot{t}")
                  for t in range(NM)]
            offs_sb = small.tile([P, E], F32, tag="offs")
            scale_all = small.tile([P, NM], F32, tag="scale")

            def evict_half(t, h):
                # (pm + be) -> fp16 SBUF (DVE; AllGather-independent)
                nc.vector.tensor_tensor(
                    ot[t][:, h * HF:(h + 1) * HF], pm[(t, h)][:],
                    be_bc[:, h * HF:(h + 1) * HF], ALU.add)

            def scale_tile(t, eng):
                sc = scale_all[:, t:t + 1]
                if eng is nc.vector:
                    nc.vector.tensor_scalar(
                        ot[t][:], ot[t][:], sc, None, ALU.mult)
                else:
                    nc.scalar.activation(ot[t][:], ot[t][:], ACT_COPY,
                                         scale=sc)

            def fused_evict_scale(t):
                # bias already folded in PSUM; halves on DVE + ACT in parallel
                sc = scale_all[:, t:t + 1]
                nc.vector.tensor_scalar(
                    ot[t][:, 0:HF], pm[(t, 0)][:], sc, None, ALU.mult)
                nc.scalar.activation(
                    ot[t][:, HF:D], pm[(t, 1)][:], ACT_COPY, scale=sc)

            def store_tile(t, eng):
                eng.dma_start(out_d[t * P:(t + 1) * P, :], ot[t][:])

            # pass-1 tiles evict early (pre-collective window)
            for t in (0, 1, 2):
                evict_half(t, 0)
                evict_half(t, 1)

            # kept mask + per-token scale (DVE)
            ag3 = agout_bc[:].rearrange("p (c e) -> p c e", e=E)
            wp = wpre_bc[:]
            wp3 = bass.AP(wp.tensor, wp.offset,
                          [wp.ap[0], [1, N_CORES], [0, E]])
            agm = small.tile([P, N_CORES * E], F32, tag="agm")
            nc.vector.tensor_tensor(
                agm[:].rearrange("p (c e) -> p c e", e=E), ag3, wp3, ALU.mult)
            am = agm[:]
            nc.vector.tensor_reduce(
                offs_sb[:],
                bass.AP(am.tensor, am.offset,
                        [am.ap[0], [1, E], [E, N_CORES]]),
                AXL.X, ALU.add)
            of = offs_sb[:]
            offs_b = bass.AP(of.tensor, of.offset, [of.ap[0], [0, NM], [1, E]])
            moff = small.tile([P, NM * E], F32, tag="moff")
            nc.vector.tensor_tensor(
                moff[:].rearrange("p (b e) -> p b e", e=E),
                mask_all[:].rearrange("p (b e) -> p b e", e=E),
                offs_b, ALU.mult)
            myoffs = small.tile([P, NM], F32, tag="myoffs")
            nc.vector.tensor_reduce(
                myoffs[:], moff[:].rearrange("p (b e) -> p b e", e=E),
                AXL.X, ALU.add)
            mypos = small.tile([P, NM], F32, tag="mypos")
            nc.vector.tensor_tensor(mypos[:], myloc[:], myoffs[:], ALU.add)
            # scale = gate * (global position <= capacity)
            nc.vector.scalar_tensor_tensor(
                scale_all[:], mypos[:], float(CAP) + 0.5, gate[:],
                ALU.is_le, ALU.mult)

            # scales + stores; stores alternate SP/ACT issue queues
            scale_tile(0, nc.vector)
            store_tile(0, nc.sync)
            scale_tile(1, nc.vector)
            store_tile(1, nc.scalar)
            scale_tile(2, nc.vector)
            store_tile(2, nc.sync)

            for t in (3, 4, 5):
                evict_half(t, 0)
                evict_half(t, 1)
                scale_tile(t, nc.vector)
                store_tile(t, nc.sync if t % 2 == 1 else nc.scalar)

            # pass-3 tiles: the tail — bias folded in PSUM, fused
            # evict+scale split across DVE/ACT, minimal hops
            fused_evict_scale(6)
            store_tile(6, nc.sync)
            fused_evict_scale(7)
            nc.sync.dma_start(out_d[7 * P:8 * P, 0:HF], ot[7][:, 0:HF])
            nc.scalar.dma_start(out_d[7 * P:8 * P, HF:D], ot[7][:, HF:D])

    nc.finalize()
    return nc


_NC_CACHE = None


def kernel(x: np.ndarray, Wg: np.ndarray, We: np.ndarray,
           be: np.ndarray) -> np.ndarray:
    global _NC_CACHE
    if _NC_CACHE is None:
        _NC_CACHE = _build_nc()
    nc = _NC_CACHE

    h = np.ascontiguousarray(np.asarray(x, dtype=np.float32).reshape(N, D))
    Wg = np.ascontiguousarray(np.asarray(Wg, dtype=np.float32))
    We = np.ascontiguousarray(np.asarray(We, dtype=np.float32))
    be2 = np.ascontiguousarray(np.asarray(be, dtype=np.float32).reshape(1, D).astype(np.float16))

    hT = np.ascontiguousarray(h.T)
    ht16 = hT.astype(np.float16)
    ht_lo = (hT - ht16.astype(np.float32)).astype(np.float16)
    Wg16 = Wg.astype(np.float16)
    Wg_loS = ((Wg - Wg16.astype(np.float32)) * 4096.0).astype(np.float16)
    Wgp16 = np.ascontiguousarray(np.concatenate([Wg16, Wg_loS], axis=1))
    We16 = We.astype(np.float16)

    in_maps = []
    for c in range(N_CORES):
        wpre = np.zeros((1, N_CORES), np.float32)
        wpre[0, :c] = 1.0
        in_maps.append({
            "ht16": np.ascontiguousarray(ht16[:, c * T:(c + 1) * T]),
            "htlo": np.ascontiguousarray(ht_lo[:, c * T:(c + 1) * T]),
            "wgp16": Wgp16,
            "we16": We16,
            "be": be2,
            "wpre": wpre,
        })

    res = run_bass_kernel_spmd(nc, in_maps, core_ids=list(range(N_CORES)))
    out = np.concatenate(
        [res.results[c]["out"] for c in range(N_CORES)], axis=0)
    return out.reshape(B, S, D).astype(np.float32)
